# revision 24
# baseline (speedup 1.0000x reference)
"""Causal single-head attention on 8 TRN2 NeuronCores.

Problem: x[B=4,T=4096,D=2048] @ Wq/Wk/Wv[D,H=128] -> causal attention -> out[B,T,H].

Sharding: 2 cores per batch (4 batches x 2 = 8 cores). Within a batch, core
parity p in {0,1} owns the interleaved 128-row query blocks Q = 2j+p
(j = 0..15), which balances causal work across the pair. Every core computes
K/V projections for its full batch (no collectives needed).

The host permutes each batch's rows to [own-parity 128-blocks | other
blocks], transposes and casts to bf16, so one 16MB xT stream feeds all three
projections (Q over the first half only). The permuted causal structure is
core-independent; per-core causality lives entirely in two 128x128 mask
inputs. Per-core algorithm (all matmuls bf16 with f32 PSUM accumulation):
  phase 1: K^T[h,s], V^T[h,s], Q^T[h,t] projected per 512-column block;
           V^T transposed on PE to V[s,h] and augmented with a ones column
           (Vhat) so the AV matmul also produces the softmax denominator.
  phase 2: per 128-key chunk c, S^T[s,t] = K^T_c.T @ Q^T[:, 128*jr(c):] on
           PE, exp on ScalarE (PSUM->SBUF, bf16), causal 0/1 mask multiply
           on the first query block of each chunk.
  phase 3: per query tile j, O[t, 0:H+1] = sum_c P^T_c.T @ Vhat_c in PSUM
           (own-chunk half pre-accumulated early for late tiles);
           normalize by the ones-column sum; DMA out. Scores and AV are
           interleaved into the projection loop for overlap.
"""

import numpy as np
import ml_dtypes

B, T, D, H = 4, 4096, 2048, 128
N_CORES = 8
P = 128  # partitions

bf16 = ml_dtypes.bfloat16


def build_nc(d=D, tkv=T, h=H):
    """v4: single permuted-xT stream. Host permutes x columns to
    [own-parity 128-blocks | other-parity 128-blocks]; Q/K/V all project from
    the same tiles (Q only over the first half). Causal structure on permuted
    key chunks is core-independent (jr = c mod n_qt); per-core causality lives
    entirely in the mask inputs (own chunks: lower-tri for both cores; other
    chunks: all-zero for parity 0, all-one for parity 1)."""
    import concourse.tile as tile
    from concourse import bacc, mybir

    assert h == P
    n_d = d // P
    n_g = tkv // 512
    n_sc = tkv // P
    tq = tkv // 2
    n_qt = tq // P
    n_gq = tq // 512
    scale = 1.0 / float(np.sqrt(h))
    BF = mybir.dt.bfloat16
    F32 = mybir.dt.float32

    nc = bacc.Bacc("TRN2", target_bir_lowering=False, debug=False,
                   num_devices=N_CORES)

    xT_ext = nc.dram_tensor("xT", [d, tkv], BF, kind="ExternalInput").ap()
    wq_ext = nc.dram_tensor("wq_pre", [P, d], BF, kind="ExternalInput").ap()
    wk_ext = nc.dram_tensor("wk_pre", [P, d], BF, kind="ExternalInput").ap()
    wv_ext = nc.dram_tensor("wv_pre", [P, d], BF, kind="ExternalInput").ap()
    mown_ext = nc.dram_tensor("m_own", [P, P], BF, kind="ExternalInput").ap()
    moth_ext = nc.dram_tensor("m_oth", [P, P], BF, kind="ExternalInput").ap()
    id_ext = nc.dram_tensor("ident", [P, P], BF, kind="ExternalInput").ap()
    out_ext = nc.dram_tensor("out", [tq, h], F32, kind="ExternalOutput").ap()

    def jr(c):
        return c % n_qt

    with tile.TileContext(nc) as tc:
        with (
            tc.tile_pool(name="const", bufs=1) as const_pool,
            tc.tile_pool(name="persist", bufs=1) as persist,
            tc.tile_pool(name="xt", bufs=16) as xt_pool,
            tc.tile_pool(name="vt", bufs=2) as vt_pool,
            tc.tile_pool(name="outp", bufs=3) as out_pool,
            tc.tile_pool(name="ps512", bufs=6, space="PSUM") as ps512,
            tc.tile_pool(name="pssm", bufs=2, space="PSUM") as pssm,
        ):
            # --- constants (wk/wv up front; wq and masks stream in between
            # the first xt tiles so the PE can start ~6us earlier) ---
            w_sb = {}
            for name, ext in (("wk", wk_ext), ("wv", wv_ext), ("wq", wq_ext)):
                t_ = const_pool.tile([P, n_d * h], BF, tag=f"w_{name}", name=name)
                if name != "wq":
                    nc.sync.dma_start(t_[:], ext[:])
                w_sb[name] = t_
            m_sb = const_pool.tile([P, 2 * P], BF, tag="masks")
            id_sb = const_pool.tile([P, P], BF, tag="ident")

            def emit_late_consts(di):
                if di == 0:
                    nc.sync.dma_start(w_sb["wq"][:], wq_ext[:])
                elif di == 1:
                    nc.sync.dma_start(m_sb[:, 0:P], mown_ext[:])
                    nc.sync.dma_start(m_sb[:, P:2 * P], moth_ext[:])
                    nc.sync.dma_start(id_sb[:], id_ext[:])

            # --- persistent activations ---
            kt_all = persist.tile([P, tkv], BF, tag="kt")
            qt_all = persist.tile([P, tq], BF, tag="qt")
            vhat = []
            for c in range(n_sc):
                vh = persist.tile([P, h + 1], BF, tag=f"vhat{c}", name=f"vh{c}")
                nc.gpsimd.memset(vh[:, h:h + 1], 1.0)
                vhat.append(vh)
            pt = [persist.tile([P, (n_qt - jr(c)) * P], BF, tag=f"pt{c}",
                               name=f"pt{c}")
                  for c in range(n_sc)]

            def emit_score_block(c, b):
                # scores for chunk c against qt 512-block b; must only be
                # emitted AFTER the qt block-b copy is emitted (Tile deps
                # follow program order)
                q0 = P * jr(c)
                t0 = max(q0, 512 * b)
                w = 512 * (b + 1) - t0
                if w <= 0:
                    return
                st_ps = ps512.tile([P, w], F32, tag="mm512", name="st_ps")
                nc.tensor.matmul(st_ps[:], kt_all[:, P * c:P * (c + 1)],
                                 qt_all[:, t0:t0 + w], start=True, stop=True)
                nc.scalar.activation(pt[c][:, t0 - q0:t0 - q0 + w], st_ps[:],
                                     mybir.ActivationFunctionType.Exp,
                                     scale=scale)
                if t0 == q0:
                    # first block of this chunk: apply the causal mask
                    msk = m_sb[:, 0:P] if c < n_sc // 2 else m_sb[:, P:2 * P]
                    nc.vector.tensor_mul(pt[c][:, 0:P], pt[c][:, 0:P], msk)

            o_part = {}

            def emit_av_own(j):
                # partial AV over own chunks, staged to SBUF; only used for
                # late query tiles to shorten the serial tail
                o_ps = pssm.tile([P, h + 1], F32, tag="small", name="o_ps")
                for ci in range(j + 1):
                    r = j - ci
                    nc.tensor.matmul(o_ps[:], pt[ci][:, P * r:P * (r + 1)],
                                     vhat[ci][:], start=(ci == 0),
                                     stop=(ci == j))
                stage = persist.tile([P, h + 1], F32, tag=f"opart{j}",
                                     name=f"opart{j}")
                nc.vector.tensor_copy(stage[:], o_ps[:])
                o_part[j] = stage

            def emit_av(j):
                o_ps = pssm.tile([P, h + 1], F32, tag="small", name="o_ps")
                if j in o_part:
                    chunks = [n_sc // 2 + i for i in range(j + 1)]
                else:
                    chunks = [i for i in range(j + 1)] + \
                             [n_sc // 2 + i for i in range(j + 1)]
                for ci, c in enumerate(chunks):
                    r = j - jr(c)
                    nc.tensor.matmul(o_ps[:], pt[c][:, P * r:P * (r + 1)],
                                     vhat[c][:], start=(ci == 0),
                                     stop=(ci == len(chunks) - 1))
                recip = out_pool.tile([P, 1], F32, tag="recip", name="recip")
                o_sum = out_pool.tile([P, h + 1], F32, tag="osum", name="o_sum")
                if j in o_part:
                    nc.vector.tensor_add(o_sum[:], o_ps[:], o_part[j][:])
                else:
                    nc.vector.tensor_copy(o_sum[:], o_ps[:])
                nc.vector.reciprocal(recip[:], o_sum[:, h:h + 1])
                o_sb = out_pool.tile([P, h], F32, tag="osb", name="o_sb")
                nc.vector.tensor_scalar_mul(o_sb[:], o_sum[:, 0:h], recip[:])
                nc.sync.dma_start(out_ext[P * j:P * (j + 1), :], o_sb[:])

            # --- phase 1: one pass over permuted xT; K/V always, Q for the
            # own half; scores/AV interleaved as dependencies are emitted ---
            av_done = 0
            for g in range(n_g):
                kt_ps = ps512.tile([P, 512], F32, tag="mm512", name="kt_ps")
                vt_ps = ps512.tile([P, 512], F32, tag="mm512", name="vt_ps")
                q_ps = (ps512.tile([P, 512], F32, tag="mm512", name="q_ps")
                        if g < n_gq else None)
                for di in range(n_d):
                    xt = xt_pool.tile([P, 512], BF, tag="xt", name="xt")
                    nc.sync.dma_start(
                        xt[:], xT_ext[di * P:(di + 1) * P, 512 * g:512 * (g + 1)])
                    if g == 0:
                        emit_late_consts(di)
                    nc.tensor.matmul(kt_ps[:], w_sb["wk"][:, di * h:(di + 1) * h],
                                     xt[:], start=(di == 0), stop=(di == n_d - 1))
                    nc.tensor.matmul(vt_ps[:], w_sb["wv"][:, di * h:(di + 1) * h],
                                     xt[:], start=(di == 0), stop=(di == n_d - 1))
                    if q_ps is not None:
                        nc.tensor.matmul(q_ps[:],
                                         w_sb["wq"][:, di * h:(di + 1) * h],
                                         xt[:], start=(di == 0),
                                         stop=(di == n_d - 1))
                nc.scalar.copy(kt_all[:, 512 * g:512 * (g + 1)], kt_ps[:])
                if q_ps is not None:
                    nc.scalar.copy(qt_all[:, 512 * g:512 * (g + 1)], q_ps[:])
                    # qt block g just became valid: emit the deferred score
                    # blocks (b == g) of all previously-emitted chunks
                    for c in range(4 * g):
                        emit_score_block(c, g)
                vt_sb = vt_pool.tile([P, 512], BF, tag="vt", name="vt_sb")
                nc.scalar.copy(vt_sb[:], vt_ps[:])
                for i in range(4):
                    c = 4 * g + i
                    vch_ps = pssm.tile([P, P], BF, tag="small", name="vch_ps")
                    nc.tensor.transpose(vch_ps[:], vt_sb[:, P * i:P * (i + 1)],
                                        id_sb[:])
                    nc.vector.tensor_copy(vhat[c][:, 0:h], vch_ps[:])
                    # emit all score blocks whose qt block already exists
                    for b in range(min(g, n_gq - 1) + 1):
                        emit_score_block(c, b)
                # late query tiles: pre-accumulate the own-chunk half as soon
                # as those chunks exist, so only the other half remains at the
                # tail
                if g < n_gq:
                    for j in range(4 * g, 4 * g + 4):
                        if j >= 8:
                            emit_av_own(j)
                # AV(j) needs own chunks 0..j (ready once g covers chunk j,
                # i.e. j <= 4g+3) and other chunks n_sc/2..n_sc/2+j
                # (ready once 4g+3 >= n_sc/2 + j)
                while av_done < n_qt and n_sc // 2 + av_done <= 4 * g + 3:
                    emit_av(av_done)
                    av_done += 1
            while av_done < n_qt:
                emit_av(av_done)
                av_done += 1

    nc.compile()
    return nc


def build_nc_v3(d=D, tkv=T, h=H, n_cores=N_CORES):
    """v3: each core projects K/V only for its own half of the keys from xqT
    (same tiles as Q), then the core pair exchanges K^T/V^T halves with an
    AllGather. Cuts input DMA from 24MB to 8MB and K/V projection PE time in
    half. The gather is split in two so scores on the first half overlap the
    second collective."""
    import concourse.tile as tile
    from concourse import bacc, mybir

    assert h == P
    n_d = d // P
    n_sc = tkv // P
    tq = tkv // 2
    n_qt = tq // P
    n_gq = tq // 512
    assert n_gq % 2 == 0
    scale = 1.0 / float(np.sqrt(h))
    BF = mybir.dt.bfloat16
    F32 = mybir.dt.float32
    half_cols = tq // 2  # own columns per gather half

    nc = bacc.Bacc("TRN2", target_bir_lowering=False, debug=False,
                   num_devices=n_cores)

    xqT_ext = nc.dram_tensor("xqT", [d, tq], BF, kind="ExternalInput").ap()
    wq_ext = nc.dram_tensor("wq_pre", [P, d], BF, kind="ExternalInput").ap()
    wk_ext = nc.dram_tensor("wk_pre", [P, d], BF, kind="ExternalInput").ap()
    wv_ext = nc.dram_tensor("wv_pre", [P, d], BF, kind="ExternalInput").ap()
    me_ext = nc.dram_tensor("m_even", [P, P], BF, kind="ExternalInput").ap()
    mo_ext = nc.dram_tensor("m_odd", [P, P], BF, kind="ExternalInput").ap()
    id_ext = nc.dram_tensor("ident", [P, P], BF, kind="ExternalInput").ap()
    out_ext = nc.dram_tensor("out", [tq, h], F32, kind="ExternalOutput").ap()

    groups = [[2 * i, 2 * i + 1] for i in range(n_cores // 2)]

    def gpos(c):
        """Column of global key chunk c inside kt_gath/vt_gath.

        Gathered layout: [r0 half0 | r1 half0 | r0 half1 | r1 half1], where
        rank r owns global 128-blocks {2i + r}."""
        pi = c % 2
        half = 0 if c < n_sc // 2 else 1
        i2 = (c - pi) // 2 - (n_qt // 2) * half
        return half_cols * (2 * half + pi) + P * i2

    with tile.TileContext(nc) as tc:
        with (
            tc.tile_pool(name="const", bufs=1) as const_pool,
            tc.tile_pool(name="persist", bufs=1) as persist,
            tc.tile_pool(name="xt", bufs=16) as xt_pool,
            tc.tile_pool(name="kv", bufs=2) as kv_pool,
            tc.tile_pool(name="outp", bufs=3) as out_pool,
            tc.tile_pool(name="dram", bufs=1, space="DRAM") as dram_pool,
            tc.tile_pool(name="ps512", bufs=4, space="PSUM") as ps512,
            tc.tile_pool(name="pssm", bufs=2, space="PSUM") as pssm,
        ):
            # --- constants (wk first: the first matmuls need it) ---
            w_sb = {}
            for name, ext in (("wk", wk_ext), ("wv", wv_ext), ("wq", wq_ext)):
                t_ = const_pool.tile([P, n_d * h], BF, tag=f"w_{name}", name=name)
                nc.sync.dma_start(t_[:], ext[:])
                w_sb[name] = t_
            m_sb = const_pool.tile([P, 2 * P], BF, tag="masks")
            nc.sync.dma_start(m_sb[:, 0:P], me_ext[:])
            nc.sync.dma_start(m_sb[:, P:2 * P], mo_ext[:])
            id_sb = const_pool.tile([P, P], BF, tag="ident")
            nc.sync.dma_start(id_sb[:], id_ext[:])

            # --- persistent ---
            kt_gath = persist.tile([P, tkv], BF, tag="ktg")
            vt_gath = persist.tile([P, tkv], BF, tag="vtg")
            qt_all = persist.tile([P, tq], BF, tag="qt")
            vhat = []
            for c in range(n_sc):
                vh = persist.tile([P, h + 1], BF, tag=f"vhat{c}", name=f"vh{c}")
                nc.gpsimd.memset(vh[:, h:h + 1], 1.0)
                vhat.append(vh)
            pt = [persist.tile([P, (n_qt - (c // 2)) * P], BF, tag=f"pt{c}",
                               name=f"pt{c}")
                  for c in range(n_sc)]

            # DRAM bounce buffers (2 halves x {in, out})
            kvin = [dram_pool.tile([2, P, half_cols], BF, tag=f"kvin{i}",
                                   name=f"kvin{i}") for i in range(2)]
            kvout = [dram_pool.tile([4, P, half_cols], BF, tag=f"kvout{i}",
                                    name=f"kvout{i}") for i in range(2)]

            def emit_scores(c):
                q0 = P * (c // 2)
                for t0 in range(q0, tq, 512):
                    w = min(512, tq - t0)
                    st_ps = ps512.tile([P, w], F32, tag="mm512", name="st_ps")
                    nc.tensor.matmul(st_ps[:], kt_gath[:, gpos(c):gpos(c) + P],
                                     qt_all[:, t0:t0 + w], start=True, stop=True)
                    nc.scalar.activation(pt[c][:, t0 - q0:t0 - q0 + w], st_ps[:],
                                         mybir.ActivationFunctionType.Exp,
                                         scale=scale)
                msk = m_sb[:, 0:P] if c % 2 == 0 else m_sb[:, P:2 * P]
                nc.vector.tensor_mul(pt[c][:, 0:P], pt[c][:, 0:P], msk)

            def emit_vhat(c):
                vch_ps = pssm.tile([P, P], BF, tag="smallbf", name="vch_ps")
                nc.tensor.transpose(vch_ps[:], vt_gath[:, gpos(c):gpos(c) + P],
                                    id_sb[:])
                nc.vector.tensor_copy(vhat[c][:, 0:h], vch_ps[:])

            def emit_av(j):
                o_ps = pssm.tile([P, h + 1], F32, tag="small", name="o_ps")
                n_c = 2 * j + 2
                for c in range(n_c):
                    r = j - (c // 2)
                    nc.tensor.matmul(o_ps[:], pt[c][:, P * r:P * (r + 1)],
                                     vhat[c][:], start=(c == 0),
                                     stop=(c == n_c - 1))
                recip = out_pool.tile([P, 1], F32, tag="recip", name="recip")
                nc.vector.reciprocal(recip[:], o_ps[:, h:h + 1])
                o_sb = out_pool.tile([P, h], F32, tag="osb", name="o_sb")
                nc.vector.tensor_scalar_mul(o_sb[:], o_ps[:, 0:h], recip[:])
                nc.sync.dma_start(out_ext[P * j:P * (j + 1), :], o_sb[:])

            # --- phase 1: Q/K/V projection from own columns, one xqT pass ---
            for g in range(n_gq):
                half, goff = g // (n_gq // 2), g % (n_gq // 2)
                q_ps = ps512.tile([P, 512], F32, tag="mm512", name="q_ps")
                kt_ps = ps512.tile([P, 512], F32, tag="mm512", name="kt_ps")
                vt_ps = ps512.tile([P, 512], F32, tag="mm512", name="vt_ps")
                for di in range(n_d):
                    xq = xt_pool.tile([P, 512], BF, tag="xt", name="xq")
                    nc.sync.dma_start(
                        xq[:], xqT_ext[di * P:(di + 1) * P, 512 * g:512 * (g + 1)])
                    nc.tensor.matmul(kt_ps[:], w_sb["wk"][:, di * h:(di + 1) * h],
                                     xq[:], start=(di == 0), stop=(di == n_d - 1))
                    nc.tensor.matmul(vt_ps[:], w_sb["wv"][:, di * h:(di + 1) * h],
                                     xq[:], start=(di == 0), stop=(di == n_d - 1))
                    nc.tensor.matmul(q_ps[:], w_sb["wq"][:, di * h:(di + 1) * h],
                                     xq[:], start=(di == 0), stop=(di == n_d - 1))
                nc.scalar.copy(qt_all[:, 512 * g:512 * (g + 1)], q_ps[:])
                kt_sb = kv_pool.tile([P, 512], BF, tag="ktsb", name="kt_sb")
                vt_sb = kv_pool.tile([P, 512], BF, tag="vtsb", name="vt_sb")
                nc.scalar.copy(kt_sb[:], kt_ps[:])
                nc.scalar.copy(vt_sb[:], vt_ps[:])
                nc.sync.dma_start(kvin[half][0, :, 512 * goff:512 * (goff + 1)],
                                  kt_sb[:])
                nc.sync.dma_start(kvin[half][1, :, 512 * goff:512 * (goff + 1)],
                                  vt_sb[:])
                if g % (n_gq // 2) == n_gq // 2 - 1:
                    # this half's K/V fully bounced out -> gather it
                    nc.gpsimd.collective_compute(
                        "AllGather", mybir.AluOpType.bypass,
                        replica_groups=groups,
                        ins=[kvin[half][:].opt()],
                        outs=[kvout[half][:].opt()],
                    )
                    base = half_cols * 2 * half
                    for r in range(2):  # rank
                        nc.sync.dma_start(
                            kt_gath[:, base + half_cols * r:
                                    base + half_cols * (r + 1)],
                            kvout[half][2 * r, :, :])
                        nc.sync.dma_start(
                            vt_gath[:, base + half_cols * r:
                                    base + half_cols * (r + 1)],
                            kvout[half][2 * r + 1, :, :])

            # --- phases 2+3, ordered so first-half chunks go first ---
            av_done = 0
            for c in range(n_sc):
                emit_vhat(c)
                emit_scores(c)
                while av_done < n_qt and 2 * av_done + 1 <= c:
                    emit_av(av_done)
                    av_done += 1
            while av_done < n_qt:
                emit_av(av_done)
                av_done += 1

    nc.compile()
    return nc


_NC_CACHE = {}


KERNEL_VERSION = "v2"


def _get_nc(d, tkv, h):
    key = (d, tkv, h, KERNEL_VERSION)
    if key not in _NC_CACHE:
        builder = build_nc_v3 if KERNEL_VERSION == "v3" else build_nc
        _NC_CACHE[key] = builder(d, tkv, h)
    return _NC_CACHE[key]


def make_in_maps(x, Wq, Wk, Wv):
    """Shard full inputs into per-core input maps (host-side prep)."""
    x = np.asarray(x, dtype=np.float32)
    b_, t_, d_ = x.shape
    tq = t_ // 2
    n_qt = tq // P
    wq = np.asarray(Wq, dtype=np.float32).astype(bf16)
    wk = np.asarray(Wk, dtype=np.float32).astype(bf16)
    wv = np.asarray(Wv, dtype=np.float32).astype(bf16)

    def prearrange(w):
        # w_pre[p, n*h + j] = w[n*128 + p, j] -> matches the SBUF layout so the
        # weight DMA is a single contiguous transfer
        n_d = w.shape[0] // P
        return np.ascontiguousarray(
            w.reshape(n_d, P, w.shape[1]).transpose(1, 0, 2).reshape(P, -1))

    wq_pre, wk_pre, wv_pre = prearrange(wq), prearrange(wk), prearrange(wv)
    tri = (np.arange(P)[None, :] >= np.arange(P)[:, None])  # [s,t]: t>=s
    t0m = tri.astype(bf16)
    ones = np.ones((P, P), dtype=bf16)
    zeros = np.zeros((P, P), dtype=bf16)
    ident = np.eye(P, dtype=bf16)
    in_maps = []
    for core in range(2 * b_):
        b, p = core // 2, core % 2
        xb16 = x[b].astype(bf16)  # [T, D]
        # permute rows: own-parity 128-blocks first, then the others
        xbb = xb16.reshape(t_ // P, P, d_)
        xperm = np.concatenate([xbb[p::2], xbb[1 - p::2]], axis=0)
        xT_perm = np.ascontiguousarray(xperm.reshape(t_, d_).T)  # [D, T]
        in_maps.append({
            "xT": xT_perm,
            "wq_pre": wq_pre, "wk_pre": wk_pre, "wv_pre": wv_pre,
            "m_own": t0m,
            "m_oth": zeros if p == 0 else ones,
            "ident": ident,
        })
    return in_maps


def gather_out(results, b_=B, t_=T, h_=H):
    """Re-interleave per-core outputs into the full [B,T,H] tensor."""
    out = np.empty((b_, t_, h_), dtype=np.float32)
    n_blocks = t_ // P
    for core in range(2 * b_):
        b, p = core // 2, core % 2
        loc = results[core]["out"].reshape(n_blocks // 2, P, h_)
        out.reshape(b_, n_blocks, P, h_)[b, p::2] = loc
    return out


def kernel(x, Wq, Wk, Wv):
    from concourse.bass_utils import run_bass_kernel_spmd

    nc = _get_nc(D, T, H)
    in_maps = make_in_maps(x, Wq, Wk, Wv)
    res = run_bass_kernel_spmd(nc, in_maps, core_ids=list(range(N_CORES)))
    return gather_out(res.results)


# revision 33
# speedup vs baseline: 1.1355x; 1.1355x over previous
"""Causal single-head attention on 8 TRN2 NeuronCores.

Problem: x[B=4,T=4096,D=2048] @ Wq/Wk/Wv[D,H=128] -> causal attention -> out[B,T,H].

Sharding: 2 cores per batch (4 batches x 2 = 8 cores). Within a batch, core
parity p in {0,1} owns the interleaved 128-row query blocks Q = 2j+p
(j = 0..15), which balances causal work across the pair. Every core computes
K/V projections for its full batch (no collectives needed).

The host permutes each batch's rows to [own-parity 128-blocks | other
blocks], transposes and casts to bf16, so one 16MB xT stream feeds all three
projections (Q over the first half only). The permuted causal structure is
core-independent; per-core causality lives entirely in two 128x128 mask
inputs. Per-core algorithm (all matmuls bf16 with f32 PSUM accumulation):
  phase 1: K^T[h,s], V^T[h,s], Q^T[h,t] projected per 512-column block;
           V^T transposed on PE to V[s,h] and augmented with a ones column
           (Vhat) so the AV matmul also produces the softmax denominator.
  phase 2: per 128-key chunk c, S^T[s,t] = K^T_c.T @ Q^T[:, 128*jr(c):] on
           PE, exp on ScalarE (PSUM->SBUF, bf16), causal 0/1 mask multiply
           on the first query block of each chunk.
  phase 3: per query tile j, O[t, 0:H+1] = sum_c P^T_c.T @ Vhat_c in PSUM
           (own-chunk half pre-accumulated early for late tiles);
           normalize by the ones-column sum; DMA out. Scores and AV are
           interleaved into the projection loop for overlap.
"""

import numpy as np
import ml_dtypes

B, T, D, H = 4, 4096, 2048, 128
N_CORES = 8
P = 128  # partitions

bf16 = ml_dtypes.bfloat16


def build_nc(d=D, tkv=T, h=H):
    """v4: single permuted-xT stream. Host permutes x columns to
    [own-parity 128-blocks | other-parity 128-blocks]; Q/K/V all project from
    the same tiles (Q only over the first half). Causal structure on permuted
    key chunks is core-independent (jr = c mod n_qt); per-core causality lives
    entirely in the mask inputs (own chunks: lower-tri for both cores; other
    chunks: all-zero for parity 0, all-one for parity 1)."""
    import concourse.tile as tile
    from concourse import bacc, mybir

    assert h == P
    n_d = d // P
    n_g = tkv // 512
    n_sc = tkv // P
    tq = tkv // 2
    n_qt = tq // P
    n_gq = tq // 512
    scale = 1.0 / float(np.sqrt(h))
    BF = mybir.dt.bfloat16
    F32 = mybir.dt.float32

    nc = bacc.Bacc("TRN2", target_bir_lowering=False, debug=False,
                   num_devices=N_CORES)

    xT_ext = nc.dram_tensor("xT", [d, tkv], BF, kind="ExternalInput").ap()
    wq_ext = nc.dram_tensor("wq_pre", [P, d], BF, kind="ExternalInput").ap()
    wk_ext = nc.dram_tensor("wk_pre", [P, d], BF, kind="ExternalInput").ap()
    wv_ext = nc.dram_tensor("wv_pre", [P, d], BF, kind="ExternalInput").ap()
    mown_ext = nc.dram_tensor("m_own", [P, P], BF, kind="ExternalInput").ap()
    moth_ext = nc.dram_tensor("m_oth", [P, P], BF, kind="ExternalInput").ap()
    id_ext = nc.dram_tensor("ident", [P, P], BF, kind="ExternalInput").ap()
    out_ext = nc.dram_tensor("out", [tq, h], F32, kind="ExternalOutput").ap()

    def jr(c):
        return c % n_qt

    with tile.TileContext(nc) as tc:
        with (
            tc.tile_pool(name="const", bufs=1) as const_pool,
            tc.tile_pool(name="persist", bufs=1) as persist,
            tc.tile_pool(name="xt", bufs=20) as xt_pool,
            tc.tile_pool(name="vt", bufs=2) as vt_pool,
            tc.tile_pool(name="outp", bufs=3) as out_pool,
            tc.tile_pool(name="ps512", bufs=2, space="PSUM") as ps512,
            tc.tile_pool(name="pssm", bufs=2, space="PSUM") as pssm,
        ):
            # --- constants (only wk up front; wv/wq/masks stream in between
            # the first xt tiles so the PE can start earlier) ---
            w_sb = {}
            for name, ext in (("wk", wk_ext), ("wv", wv_ext), ("wq", wq_ext)):
                t_ = const_pool.tile([P, n_d * h], BF, tag=f"w_{name}", name=name)
                if name == "wk":
                    nc.sync.dma_start(t_[:], ext[:])
                w_sb[name] = t_
            m_sb = const_pool.tile([P, 2 * P], BF, tag="masks")
            id_sb = const_pool.tile([P, P], BF, tag="ident")

            def emit_late_consts(di):
                # wv/wq must be emitted at di==0, before the first V/Q matmuls
                # that read them (Tile deps follow program order)
                if di == 0:
                    nc.sync.dma_start(w_sb["wv"][:], wv_ext[:])
                    nc.sync.dma_start(w_sb["wq"][:], wq_ext[:])
                elif di == 1:
                    nc.sync.dma_start(m_sb[:, 0:P], mown_ext[:])
                    nc.sync.dma_start(m_sb[:, P:2 * P], moth_ext[:])
                    nc.sync.dma_start(id_sb[:], id_ext[:])

            # --- persistent activations ---
            kt_all = persist.tile([P, tkv], BF, tag="kt")
            qt_all = persist.tile([P, tq], BF, tag="qt")
            vhat = []
            for c in range(n_sc):
                vh = persist.tile([P, h + 1], BF, tag=f"vhat{c}", name=f"vh{c}")
                nc.gpsimd.memset(vh[:, h:h + 1], 1.0)
                vhat.append(vh)
            pt = [persist.tile([P, (n_qt - jr(c)) * P], BF, tag=f"pt{c}",
                               name=f"pt{c}")
                  for c in range(n_sc)]

            def emit_score_block(c, b):
                # scores for chunk c against qt 512-block b; must only be
                # emitted AFTER the qt block-b copy is emitted (Tile deps
                # follow program order)
                q0 = P * jr(c)
                t0 = max(q0, 512 * b)
                w = 512 * (b + 1) - t0
                if w <= 0:
                    return
                st_ps = ps512.tile([P, w], F32, tag="mm512", name="st_ps")
                nc.tensor.matmul(st_ps[:], kt_all[:, P * c:P * (c + 1)],
                                 qt_all[:, t0:t0 + w], start=True, stop=True)
                nc.scalar.activation(pt[c][:, t0 - q0:t0 - q0 + w], st_ps[:],
                                     mybir.ActivationFunctionType.Exp,
                                     scale=scale)
                if t0 == q0:
                    # first block of this chunk: apply the causal mask
                    msk = m_sb[:, 0:P] if c < n_sc // 2 else m_sb[:, P:2 * P]
                    nc.vector.tensor_mul(pt[c][:, 0:P], pt[c][:, 0:P], msk)

            o_part = {}

            def emit_av_own(j):
                # partial AV over own chunks, staged to SBUF; only used for
                # late query tiles to shorten the serial tail
                o_ps = pssm.tile([P, h + 1], F32, tag="small", name="o_ps")
                for ci in range(j + 1):
                    r = j - ci
                    nc.tensor.matmul(o_ps[:], pt[ci][:, P * r:P * (r + 1)],
                                     vhat[ci][:], start=(ci == 0),
                                     stop=(ci == j))
                stage = persist.tile([P, h + 1], F32, tag=f"opart{j}",
                                     name=f"opart{j}")
                nc.vector.tensor_copy(stage[:], o_ps[:])
                o_part[j] = stage

            def emit_av(j):
                o_ps = pssm.tile([P, h + 1], F32, tag="small", name="o_ps")
                if j in o_part:
                    chunks = [n_sc // 2 + i for i in range(j + 1)]
                else:
                    chunks = [i for i in range(j + 1)] + \
                             [n_sc // 2 + i for i in range(j + 1)]
                for ci, c in enumerate(chunks):
                    r = j - jr(c)
                    nc.tensor.matmul(o_ps[:], pt[c][:, P * r:P * (r + 1)],
                                     vhat[c][:], start=(ci == 0),
                                     stop=(ci == len(chunks) - 1))
                recip = out_pool.tile([P, 1], F32, tag="recip", name="recip")
                o_sum = out_pool.tile([P, h + 1], F32, tag="osum", name="o_sum")
                if j in o_part:
                    nc.vector.tensor_add(o_sum[:], o_ps[:], o_part[j][:])
                else:
                    nc.vector.tensor_copy(o_sum[:], o_ps[:])
                nc.vector.reciprocal(recip[:], o_sum[:, h:h + 1])
                o_sb = out_pool.tile([P, h], F32, tag="osb", name="o_sb")
                nc.vector.tensor_scalar_mul(o_sb[:], o_sum[:, 0:h], recip[:])
                nc.sync.dma_start(out_ext[P * j:P * (j + 1), :], o_sb[:])

            # --- phase 1: one pass over permuted xT; K/V always, Q for the
            # own half; scores/AV interleaved as dependencies are emitted ---
            av_done = 0
            for g in range(n_g):
                kt_ps = ps512.tile([P, 512], F32, tag="acc", bufs=4, name="kt_ps")
                vt_ps = ps512.tile([P, 512], F32, tag="acc", bufs=4, name="vt_ps")
                q_ps = (ps512.tile([P, 512], F32, tag="acc", bufs=4, name="q_ps")
                        if g < n_gq else None)
                if g == 0:
                    # split loops: all K matmuls first (they only need wk,
                    # which is the only weight loaded up front) so the PE has
                    # work while wv/wq stream in between the first xt tiles
                    tiles = []
                    for di in range(n_d):
                        xt = xt_pool.tile([P, 512], BF, tag="xt", name="xt")
                        nc.sync.dma_start(
                            xt[:], xT_ext[di * P:(di + 1) * P, 0:512])
                        emit_late_consts(di)
                        tiles.append(xt)
                        nc.tensor.matmul(kt_ps[:],
                                         w_sb["wk"][:, di * h:(di + 1) * h],
                                         xt[:], start=(di == 0),
                                         stop=(di == n_d - 1))
                    for di, xt in enumerate(tiles):
                        nc.tensor.matmul(vt_ps[:],
                                         w_sb["wv"][:, di * h:(di + 1) * h],
                                         xt[:], start=(di == 0),
                                         stop=(di == n_d - 1))
                    for di, xt in enumerate(tiles):
                        nc.tensor.matmul(q_ps[:],
                                         w_sb["wq"][:, di * h:(di + 1) * h],
                                         xt[:], start=(di == 0),
                                         stop=(di == n_d - 1))
                else:
                    for di in range(n_d):
                        xt = xt_pool.tile([P, 512], BF, tag="xt", name="xt")
                        nc.sync.dma_start(
                            xt[:], xT_ext[di * P:(di + 1) * P,
                                          512 * g:512 * (g + 1)])
                        nc.tensor.matmul(kt_ps[:],
                                         w_sb["wk"][:, di * h:(di + 1) * h],
                                         xt[:], start=(di == 0),
                                         stop=(di == n_d - 1))
                        nc.tensor.matmul(vt_ps[:],
                                         w_sb["wv"][:, di * h:(di + 1) * h],
                                         xt[:], start=(di == 0),
                                         stop=(di == n_d - 1))
                        if q_ps is not None:
                            nc.tensor.matmul(q_ps[:],
                                             w_sb["wq"][:, di * h:(di + 1) * h],
                                             xt[:], start=(di == 0),
                                             stop=(di == n_d - 1))
                nc.vector.tensor_copy(kt_all[:, 512 * g:512 * (g + 1)], kt_ps[:])
                if q_ps is not None:
                    nc.vector.tensor_copy(qt_all[:, 512 * g:512 * (g + 1)], q_ps[:])
                    # qt block g just became valid: emit the deferred score
                    # blocks (b == g) of all previously-emitted chunks
                    for c in range(4 * g):
                        emit_score_block(c, g)
                vt_sb = vt_pool.tile([P, 512], BF, tag="vt", name="vt_sb")
                nc.vector.tensor_copy(vt_sb[:], vt_ps[:])
                for i in range(4):
                    c = 4 * g + i
                    vch_ps = pssm.tile([P, P], BF, tag="small", name="vch_ps")
                    nc.tensor.transpose(vch_ps[:], vt_sb[:, P * i:P * (i + 1)],
                                        id_sb[:])
                    nc.vector.tensor_copy(vhat[c][:, 0:h], vch_ps[:])
                    # emit all score blocks whose qt block already exists
                    for b in range(min(g, n_gq - 1) + 1):
                        emit_score_block(c, b)
                # late query tiles: pre-accumulate the own-chunk half as soon
                # as those chunks exist, so only the other half remains at the
                # tail
                if g < n_gq:
                    for j in range(4 * g, 4 * g + 4):
                        if j >= 8:
                            emit_av_own(j)
                # AV(j) needs own chunks 0..j (ready once g covers chunk j,
                # i.e. j <= 4g+3) and other chunks n_sc/2..n_sc/2+j
                # (ready once 4g+3 >= n_sc/2 + j)
                while av_done < n_qt and n_sc // 2 + av_done <= 4 * g + 3:
                    emit_av(av_done)
                    av_done += 1
            while av_done < n_qt:
                emit_av(av_done)
                av_done += 1

    nc.compile()
    return nc


_NC_CACHE = {}


def _get_nc(d, tkv, h):
    key = (d, tkv, h)
    if key not in _NC_CACHE:
        _NC_CACHE[key] = build_nc(d, tkv, h)
    return _NC_CACHE[key]


def make_in_maps(x, Wq, Wk, Wv):
    """Shard full inputs into per-core input maps (host-side prep)."""
    x = np.asarray(x, dtype=np.float32)
    b_, t_, d_ = x.shape
    tq = t_ // 2
    n_qt = tq // P
    wq = np.asarray(Wq, dtype=np.float32).astype(bf16)
    wk = np.asarray(Wk, dtype=np.float32).astype(bf16)
    wv = np.asarray(Wv, dtype=np.float32).astype(bf16)

    def prearrange(w):
        # w_pre[p, n*h + j] = w[n*128 + p, j] -> matches the SBUF layout so the
        # weight DMA is a single contiguous transfer
        n_d = w.shape[0] // P
        return np.ascontiguousarray(
            w.reshape(n_d, P, w.shape[1]).transpose(1, 0, 2).reshape(P, -1))

    wq_pre, wk_pre, wv_pre = prearrange(wq), prearrange(wk), prearrange(wv)
    tri = (np.arange(P)[None, :] >= np.arange(P)[:, None])  # [s,t]: t>=s
    t0m = tri.astype(bf16)
    ones = np.ones((P, P), dtype=bf16)
    zeros = np.zeros((P, P), dtype=bf16)
    ident = np.eye(P, dtype=bf16)
    in_maps = []
    for core in range(2 * b_):
        b, p = core // 2, core % 2
        xb16 = x[b].astype(bf16)  # [T, D]
        # permute rows: own-parity 128-blocks first, then the others
        xbb = xb16.reshape(t_ // P, P, d_)
        xperm = np.concatenate([xbb[p::2], xbb[1 - p::2]], axis=0)
        xT_perm = np.ascontiguousarray(xperm.reshape(t_, d_).T)  # [D, T]
        in_maps.append({
            "xT": xT_perm,
            "wq_pre": wq_pre, "wk_pre": wk_pre, "wv_pre": wv_pre,
            "m_own": t0m,
            "m_oth": zeros if p == 0 else ones,
            "ident": ident,
        })
    return in_maps


def gather_out(results, b_=B, t_=T, h_=H):
    """Re-interleave per-core outputs into the full [B,T,H] tensor."""
    out = np.empty((b_, t_, h_), dtype=np.float32)
    n_blocks = t_ // P
    for core in range(2 * b_):
        b, p = core // 2, core % 2
        loc = results[core]["out"].reshape(n_blocks // 2, P, h_)
        out.reshape(b_, n_blocks, P, h_)[b, p::2] = loc
    return out


def kernel(x, Wq, Wk, Wv):
    from concourse.bass_utils import run_bass_kernel_spmd

    nc = _get_nc(D, T, H)
    in_maps = make_in_maps(x, Wq, Wk, Wv)
    res = run_bass_kernel_spmd(nc, in_maps, core_ids=list(range(N_CORES)))
    return gather_out(res.results)



# revision 57
# speedup vs baseline: 1.1585x; 1.0203x over previous
"""Causal single-head attention on 8 TRN2 NeuronCores.

Problem: x[B=4,T=4096,D=2048] @ Wq/Wk/Wv[D,H=128] -> causal attention -> out[B,T,H].

Sharding: 2 cores per batch (4 batches x 2 = 8 cores). Within a batch, core
parity p in {0,1} owns the interleaved 128-row query blocks Q = 2j+p
(j = 0..15), which balances causal work across the pair. Every core computes
K/V projections for its full batch (no collectives needed).

The host permutes each batch's rows to [own-parity 128-blocks | other
blocks], transposes and casts to bf16, so one 16MB xT stream feeds all three
projections (Q over the first half only). The permuted causal structure is
core-independent; per-core causality lives entirely in two 128x128 mask
inputs. Per-core algorithm (all matmuls bf16 with f32 PSUM accumulation):
  phase 1: K^T[h,s], V^T[h,s], Q^T[h,t] projected per 512-column block;
           V^T transposed on PE to V[s,h] and augmented with a ones column
           (Vhat) so the AV matmul also produces the softmax denominator.
  phase 2: per 128-key chunk c, S^T[s,t] = K^T_c.T @ Q^T[:, 128*jr(c):] on
           PE, exp on ScalarE (PSUM->SBUF, bf16), causal 0/1 mask multiply
           on the first query block of each chunk.
  phase 3: per query tile j, O[t, 0:H+1] = sum_c P^T_c.T @ Vhat_c in PSUM
           (own-chunk half pre-accumulated early for late tiles);
           normalize by the ones-column sum; DMA out. Scores and AV are
           interleaved into the projection loop for overlap.
"""

import numpy as np
import ml_dtypes

B, T, D, H = 4, 4096, 2048, 128
N_CORES = 8
P = 128  # partitions

bf16 = ml_dtypes.bfloat16


def build_nc(d=D, tkv=T, h=H):
    """v4: single permuted-xT stream. Host permutes x columns to
    [own-parity 128-blocks | other-parity 128-blocks]; Q/K/V all project from
    the same tiles (Q only over the first half). Causal structure on permuted
    key chunks is core-independent (jr = c mod n_qt); per-core causality lives
    entirely in the mask inputs (own chunks: lower-tri for both cores; other
    chunks: all-zero for parity 0, all-one for parity 1)."""
    import concourse.tile as tile
    from concourse import bacc, mybir

    assert h == P
    n_d = d // P
    n_g = tkv // 512
    n_sc = tkv // P
    tq = tkv // 2
    n_qt = tq // P
    n_gq = tq // 512
    scale = 1.0 / float(np.sqrt(h))
    BF = mybir.dt.bfloat16
    F32 = mybir.dt.float32

    nc = bacc.Bacc("TRN2", target_bir_lowering=False, debug=False,
                   num_devices=N_CORES)

    xT_ext = nc.dram_tensor("xT", [d, tkv], BF, kind="ExternalInput").ap()
    wq_ext = nc.dram_tensor("wq_pre", [P, d], BF, kind="ExternalInput").ap()
    wk_ext = nc.dram_tensor("wk_pre", [P, d], BF, kind="ExternalInput").ap()
    wv_ext = nc.dram_tensor("wv_pre", [P, d], BF, kind="ExternalInput").ap()
    # masks and identity packed into one tensor -> one DMA
    mi_ext = nc.dram_tensor("mask_id", [P, 3 * P], BF, kind="ExternalInput").ap()
    out_ext = nc.dram_tensor("out", [tq, h], F32, kind="ExternalOutput").ap()

    def jr(c):
        return c % n_qt

    with tile.TileContext(nc) as tc:
        with (
            tc.tile_pool(name="const", bufs=1) as const_pool,
            tc.tile_pool(name="persist", bufs=1) as persist,
            tc.tile_pool(name="xt", bufs=20) as xt_pool,
            tc.tile_pool(name="vt", bufs=2) as vt_pool,
            tc.tile_pool(name="outp", bufs=3) as out_pool,
            tc.tile_pool(name="ps512", bufs=2, space="PSUM") as ps512,
            tc.tile_pool(name="pssm", bufs=2, space="PSUM") as pssm,
        ):
            # --- constants (only wk up front; wv/wq/masks stream in between
            # the first xt tiles so the PE can start earlier) ---
            w_sb = {}
            for name, ext in (("wk", wk_ext), ("wv", wv_ext), ("wq", wq_ext)):
                t_ = const_pool.tile([P, n_d * h], BF, tag=f"w_{name}", name=name)
                if name == "wk":
                    # two half-DMAs: the first K matmul only reads the first
                    # columns, so it can start ~0.7us earlier
                    half = (n_d * h) // 2
                    nc.sync.dma_start(t_[:, 0:half], ext[:, 0:half])
                    nc.sync.dma_start(t_[:, half:], ext[:, half:])
                w_sb[name] = t_
            mi_sb = const_pool.tile([P, 3 * P], BF, tag="maskid")
            m_sb = mi_sb[:, 0:2 * P]
            id_sb = mi_sb[:, 2 * P:3 * P]

            def emit_late_consts(di):
                # wv/wq must be emitted at di==0, before the first V/Q matmuls
                # that read them; masks/ident are not needed until the first
                # scores (~12us), so they load behind several xt tiles
                if di == 0:
                    nc.sync.dma_start(w_sb["wv"][:], wv_ext[:])
                    nc.sync.dma_start(w_sb["wq"][:], wq_ext[:])
                elif di == min(6, n_d - 1):
                    nc.sync.dma_start(mi_sb[:], mi_ext[:])

            # --- PE warmup: a few throwaway matmuls during the DMA-bound
            # head so the p-state/HAM ramp is spent before real work ---
            warm = const_pool.tile([P, 512], BF, tag="warm")
            nc.gpsimd.memset(warm[:], 0.125)
            for _ in range(16):
                wu_ps = ps512.tile([P, 512], F32, tag="mm512", name="wu_ps")
                nc.tensor.matmul(wu_ps[:], warm[:, 0:P], warm[:],
                                 start=True, stop=True)

            # --- persistent activations ---
            kt_all = persist.tile([P, tkv], BF, tag="kt")
            qt_all = persist.tile([P, tq], BF, tag="qt")
            vhat = []
            for c in range(n_sc):
                vh = persist.tile([P, h + 1], BF, tag=f"vhat{c}", name=f"vh{c}")
                nc.gpsimd.memset(vh[:, h:h + 1], 1.0)
                vhat.append(vh)
            pt = [persist.tile([P, (n_qt - jr(c)) * P], BF, tag=f"pt{c}",
                               name=f"pt{c}")
                  for c in range(n_sc)]

            qt_blocks_ready = set()
            chunks_emitted = []
            scores_done = set()

            def _emit_score_block(c, b):
                q0 = P * jr(c)
                t0 = max(q0, 512 * b)
                w = 512 * (b + 1) - t0
                if w <= 0:
                    return
                st_ps = ps512.tile([P, w], F32, tag="mm512", name="st_ps")
                nc.tensor.matmul(st_ps[:], kt_all[:, P * c:P * (c + 1)],
                                 qt_all[:, t0:t0 + w], start=True, stop=True)
                nc.scalar.activation(pt[c][:, t0 - q0:t0 - q0 + w], st_ps[:],
                                     mybir.ActivationFunctionType.Exp,
                                     scale=scale)
                if t0 == q0:
                    # first block of this chunk: apply the causal mask
                    msk = m_sb[:, 0:P] if c < n_sc // 2 else m_sb[:, P:2 * P]
                    nc.vector.tensor_mul(pt[c][:, 0:P], pt[c][:, 0:P], msk)

            def flush_scores():
                # emit every (chunk, qt-block) pair whose inputs have been
                # emitted (Tile deps follow program order, so a score matmul
                # may only be emitted after its qt block's copy)
                for c in chunks_emitted:
                    for b in sorted(qt_blocks_ready):
                        if (c, b) not in scores_done:
                            scores_done.add((c, b))
                            _emit_score_block(c, b)

            o_part = {}

            def emit_av_own(j):
                # partial AV over own chunks, staged to SBUF; only used for
                # late query tiles to shorten the serial tail
                o_ps = pssm.tile([P, h + 1], F32, tag="small", name="o_ps")
                for ci in range(j + 1):
                    r = j - ci
                    nc.tensor.matmul(o_ps[:], pt[ci][:, P * r:P * (r + 1)],
                                     vhat[ci][:], start=(ci == 0),
                                     stop=(ci == j))
                stage = persist.tile([P, h + 1], F32, tag=f"opart{j}",
                                     name=f"opart{j}")
                nc.vector.tensor_copy(stage[:], o_ps[:])
                o_part[j] = stage

            def emit_av(j):
                o_ps = pssm.tile([P, h + 1], F32, tag="small", name="o_ps")
                if j in o_part:
                    chunks = [n_sc // 2 + i for i in range(j + 1)]
                else:
                    chunks = [i for i in range(j + 1)] + \
                             [n_sc // 2 + i for i in range(j + 1)]
                for ci, c in enumerate(chunks):
                    r = j - jr(c)
                    nc.tensor.matmul(o_ps[:], pt[c][:, P * r:P * (r + 1)],
                                     vhat[c][:], start=(ci == 0),
                                     stop=(ci == len(chunks) - 1))
                recip = out_pool.tile([P, 1], F32, tag="recip", name="recip")
                o_sum = out_pool.tile([P, h + 1], F32, tag="osum", name="o_sum")
                if j in o_part:
                    nc.vector.tensor_add(o_sum[:], o_ps[:], o_part[j][:])
                else:
                    nc.vector.tensor_copy(o_sum[:], o_ps[:])
                nc.vector.reciprocal(recip[:], o_sum[:, h:h + 1])
                o_sb = out_pool.tile([P, h], F32, tag="osb", name="o_sb")
                nc.vector.tensor_scalar_mul(o_sb[:], o_sum[:, 0:h], recip[:])
                nc.sync.dma_start(out_ext[P * j:P * (j + 1), :], o_sb[:])

            # --- phase 1: one pass over permuted xT; K/V always, Q for the
            # own half; scores/AV interleaved as dependencies are emitted ---
            def emit_block_epilogue(g, kt_ps, q_ps, vt_ps):
                nc.vector.tensor_copy(kt_all[:, 512 * g:512 * (g + 1)], kt_ps[:])
                if q_ps is not None:
                    nc.vector.tensor_copy(qt_all[:, 512 * g:512 * (g + 1)],
                                          q_ps[:])
                    qt_blocks_ready.add(g)
                    flush_scores()
                vt_sb = vt_pool.tile([P, 512], BF, tag="vt", name="vt_sb")
                nc.vector.tensor_copy(vt_sb[:], vt_ps[:])
                for i in range(4):
                    c = 4 * g + i
                    vch_ps = ps512.tile([P, P], BF, tag="mm512", name="vch_ps")
                    nc.tensor.transpose(vch_ps[:], vt_sb[:, P * i:P * (i + 1)],
                                        id_sb)
                    nc.vector.tensor_copy(vhat[c][:, 0:h], vch_ps[:])
                    chunks_emitted.append(c)
                    flush_scores()

            def emit_av_batch(g):
                nonlocal av_done
                if g < n_gq:
                    for j in range(4 * g, 4 * g + 4):
                        if j >= 8:
                            emit_av_own(j)
                while av_done < n_qt and n_sc // 2 + av_done <= 4 * g + 3:
                    emit_av(av_done)
                    av_done += 1

            av_done = 0
            # --- first half (g < n_gq): single-block loads, K/V/Q ---
            for g in range(n_gq):
                kt_ps = ps512.tile([P, 512], F32, tag="acc", bufs=4, name="kt_ps")
                vt_ps = ps512.tile([P, 512], F32, tag="acc", bufs=4, name="vt_ps")
                q_ps = ps512.tile([P, 512], F32, tag="acc", bufs=4, name="q_ps")
                if g == 0:
                    # split loops: all K matmuls first (they only need wk,
                    # which is the only weight loaded up front) so the PE has
                    # work while wv/wq stream in between the first xt tiles
                    tiles = []
                    for di in range(n_d):
                        xt = xt_pool.tile([P, 512], BF, tag="xt", name="xt")
                        nc.sync.dma_start(
                            xt[:], xT_ext[di * P:(di + 1) * P, 0:512])
                        emit_late_consts(di)
                        tiles.append(xt)
                        nc.tensor.matmul(kt_ps[:],
                                         w_sb["wk"][:, di * h:(di + 1) * h],
                                         xt[:], start=(di == 0),
                                         stop=(di == n_d - 1))
                    for di, xt in enumerate(tiles):
                        nc.tensor.matmul(vt_ps[:],
                                         w_sb["wv"][:, di * h:(di + 1) * h],
                                         xt[:], start=(di == 0),
                                         stop=(di == n_d - 1))
                    for di, xt in enumerate(tiles):
                        nc.tensor.matmul(q_ps[:],
                                         w_sb["wq"][:, di * h:(di + 1) * h],
                                         xt[:], start=(di == 0),
                                         stop=(di == n_d - 1))
                else:
                    for di in range(n_d):
                        xt = xt_pool.tile([P, 512], BF, tag="xt", name="xt")
                        nc.sync.dma_start(
                            xt[:], xT_ext[di * P:(di + 1) * P,
                                          512 * g:512 * (g + 1)])
                        nc.tensor.matmul(kt_ps[:],
                                         w_sb["wk"][:, di * h:(di + 1) * h],
                                         xt[:], start=(di == 0),
                                         stop=(di == n_d - 1))
                        nc.tensor.matmul(vt_ps[:],
                                         w_sb["wv"][:, di * h:(di + 1) * h],
                                         xt[:], start=(di == 0),
                                         stop=(di == n_d - 1))
                        nc.tensor.matmul(q_ps[:],
                                         w_sb["wq"][:, di * h:(di + 1) * h],
                                         xt[:], start=(di == 0),
                                         stop=(di == n_d - 1))
                emit_block_epilogue(g, kt_ps, q_ps, vt_ps)
                emit_av_batch(g)

            # --- second half (no Q): pair-loaded [128,1024] tiles for 2KB
            # DMA lines; K/V matmuls for both blocks share each tile ---
            for g in range(n_gq, n_g, 2):
                kps = [ps512.tile([P, 512], F32, tag="acc", bufs=4,
                                  name=f"kt_ps{gg}") for gg in (0, 1)]
                vps = [ps512.tile([P, 512], F32, tag="acc", bufs=4,
                                  name=f"vt_ps{gg}") for gg in (0, 1)]
                for di in range(n_d):
                    xt = xt_pool.tile([P, 1024], BF, tag="xt2", bufs=6,
                                      name="xt2")
                    nc.sync.dma_start(
                        xt[:], xT_ext[di * P:(di + 1) * P,
                                      512 * g:512 * (g + 2)])
                    for gg in (0, 1):
                        nc.tensor.matmul(kps[gg][:],
                                         w_sb["wk"][:, di * h:(di + 1) * h],
                                         xt[:, 512 * gg:512 * (gg + 1)],
                                         start=(di == 0), stop=(di == n_d - 1))
                        nc.tensor.matmul(vps[gg][:],
                                         w_sb["wv"][:, di * h:(di + 1) * h],
                                         xt[:, 512 * gg:512 * (gg + 1)],
                                         start=(di == 0), stop=(di == n_d - 1))
                for gg in (0, 1):
                    emit_block_epilogue(g + gg, kps[gg], None, vps[gg])
                    emit_av_batch(g + gg)
            while av_done < n_qt:
                emit_av(av_done)
                av_done += 1

    nc.compile()
    return nc


_NC_CACHE = {}


def _get_nc(d, tkv, h):
    key = (d, tkv, h)
    if key not in _NC_CACHE:
        _NC_CACHE[key] = build_nc(d, tkv, h)
    return _NC_CACHE[key]


def make_in_maps(x, Wq, Wk, Wv):
    """Shard full inputs into per-core input maps (host-side prep)."""
    x = np.asarray(x, dtype=np.float32)
    b_, t_, d_ = x.shape
    tq = t_ // 2
    n_qt = tq // P
    wq = np.asarray(Wq, dtype=np.float32).astype(bf16)
    wk = np.asarray(Wk, dtype=np.float32).astype(bf16)
    wv = np.asarray(Wv, dtype=np.float32).astype(bf16)

    def prearrange(w):
        # w_pre[p, n*h + j] = w[n*128 + p, j] -> matches the SBUF layout so the
        # weight DMA is a single contiguous transfer
        n_d = w.shape[0] // P
        return np.ascontiguousarray(
            w.reshape(n_d, P, w.shape[1]).transpose(1, 0, 2).reshape(P, -1))

    wq_pre, wk_pre, wv_pre = prearrange(wq), prearrange(wk), prearrange(wv)
    tri = (np.arange(P)[None, :] >= np.arange(P)[:, None])  # [s,t]: t>=s
    t0m = tri.astype(bf16)
    ones = np.ones((P, P), dtype=bf16)
    zeros = np.zeros((P, P), dtype=bf16)
    ident = np.eye(P, dtype=bf16)
    in_maps = []
    for core in range(2 * b_):
        b, p = core // 2, core % 2
        xb16 = x[b].astype(bf16)  # [T, D]
        # permute rows: own-parity 128-blocks first, then the others
        xbb = xb16.reshape(t_ // P, P, d_)
        xperm = np.concatenate([xbb[p::2], xbb[1 - p::2]], axis=0)
        xT_perm = np.ascontiguousarray(xperm.reshape(t_, d_).T)  # [D, T]
        mask_id = np.concatenate(
            [t0m, zeros if p == 0 else ones, ident], axis=1)
        in_maps.append({
            "xT": xT_perm,
            "wq_pre": wq_pre, "wk_pre": wk_pre, "wv_pre": wv_pre,
            "mask_id": np.ascontiguousarray(mask_id),
        })
    return in_maps


def gather_out(results, b_=B, t_=T, h_=H):
    """Re-interleave per-core outputs into the full [B,T,H] tensor."""
    out = np.empty((b_, t_, h_), dtype=np.float32)
    n_blocks = t_ // P
    for core in range(2 * b_):
        b, p = core // 2, core % 2
        loc = results[core]["out"].reshape(n_blocks // 2, P, h_)
        out.reshape(b_, n_blocks, P, h_)[b, p::2] = loc
    return out


def kernel(x, Wq, Wk, Wv):
    from concourse.bass_utils import run_bass_kernel_spmd

    nc = _get_nc(D, T, H)
    in_maps = make_in_maps(x, Wq, Wk, Wv)
    res = run_bass_kernel_spmd(nc, in_maps, core_ids=list(range(N_CORES)))
    return gather_out(res.results)

